# revision 1
# baseline (speedup 1.0000x reference)
"""Trainium2 Bass kernel for a 6-layer post-BatchNorm transformer encoder.

Reference model:
  x = emb[seq] + pes                                  # [B,S,D] = [4,512,1024]
  6x: x = BN(x + attn(x)); x = BN(x + ffn(x))
  BN = per-channel batch stats over (B,S), eps=1e-3.

Sharding: tensor-parallel across 8 NeuronCores. Each core owns H/8=2 heads
(QKV out / Wo in slices) and DF/8=512 FFN hidden units. After Wo and after
W2 an fp32 AllReduce combines partial [D, T] outputs; the residual x/8 is
folded into each partial via an extra (1/8)*I matmul so the AllReduce
directly yields x + sublayer(x). bo/b2 biases cancel inside BN and are
dropped. BatchNorm is computed redundantly on every core, keeping the
program SPMD-uniform (no rank-dependent addressing anywhere).

Activation layout: transposed. x^T lives in SBUF as [128 part, 8 dtile,
2048 tok] so natural-layout weights serve directly as matmul lhsT
(stationary) and activations as rhs (moving); no per-layer activation
transposes. Attention per (batch, head): scores^T = K_h @ Q_h^T,
E = exp(scale*scores^T) (softmax max-subtraction skipped; scores are O(1)),
U^T = V_h^T @ E^T with column sums from a ones-row matmul, normalized by a
PE-broadcast reciprocal row. Matmuls run as float32r (full-rate fp32 PE
mode; plain fp32 is 4x slower).
"""

import os

import numpy as np

import concourse.bass as bass
import concourse.mybir as mybir
import concourse.tile as tile
from concourse import bacc
from concourse.bass import ts
from concourse.masks import make_identity

# ---------------------------------------------------------------- dims
V, D, L, H, B, S = 32000, 1024, 6, 16, 4, 512
HD = D // H            # 64
DF = 4 * D             # 4096
EPS = 1e-3
NC = 8                 # cores
T = B * S              # 2048 tokens
P = 128                # partitions
DT = D // P            # 8 d-tiles
TT = T // P            # 16 token tiles
CH = 512               # token chunk (matmul N)
NCH = T // CH          # 4 chunks
HPC = H // NC          # heads per core = 2
DSH = HPC * HD         # qkv out shard = 128
FSH = DF // NC         # ffn hidden shard = 512
FMT = FSH // P         # ffn1 m-tiles = 4
KL = FSH // P          # ffn2 k-tiles = 4

f32 = mybir.dt.float32
f16 = mybir.dt.float16
f32r = mybir.dt.float32r
i16 = mybir.dt.int16
AF = mybir.ActivationFunctionType
ALU = mybir.AluOpType

REPLICAS = [list(range(NC))]

N_LAYERS = int(os.environ.get("TRN_KERNEL_LAYERS", str(L)))
DEBUG_TAPS = os.environ.get("TRN_KERNEL_DEBUG", "0") == "1"

GATHER_QUEUES = int(os.environ.get("TRN_GATHER_QUEUES", "1"))


def _r(ap):
    """view an fp32 AP as float32r for full-rate PE matmul"""
    return ap.bitcast(f32r)


def build_module(n_layers=None):
    if n_layers is None:
        n_layers = N_LAYERS
    nc = bacc.Bacc("TRN2", target_bir_lowering=False, debug=False,
                   num_devices=NC)

    dt_ = nc.dram_tensor
    io = {
        "emb": dt_("emb", [V, D], f32, kind="ExternalInput").ap(),
        "idx": dt_("idx", [16, T // 16], i16, kind="ExternalInput").ap(),
        "pesT": dt_("pesT", [D, S], f32, kind="ExternalInput").ap(),
        "wq": dt_("wq", [L, D, DSH], f32, kind="ExternalInput").ap(),
        "wk": dt_("wk", [L, D, DSH], f32, kind="ExternalInput").ap(),
        "wv": dt_("wv", [L, D, DSH], f32, kind="ExternalInput").ap(),
        "wo": dt_("wo", [L, DSH, D], f32, kind="ExternalInput").ap(),
        "w1": dt_("w1", [L, D, FSH], f32, kind="ExternalInput").ap(),
        "w2": dt_("w2", [L, FSH, D], f32, kind="ExternalInput").ap(),
        "bq": dt_("bq", [L, DSH], f32, kind="ExternalInput").ap(),
        "bk": dt_("bk", [L, DSH], f32, kind="ExternalInput").ap(),
        "bv": dt_("bv", [L, DSH], f32, kind="ExternalInput").ap(),
        "b1": dt_("b1", [L, FSH], f32, kind="ExternalInput").ap(),
        "g1": dt_("g1", [L, D], f32, kind="ExternalInput").ap(),
        "be1": dt_("be1", [L, D], f32, kind="ExternalInput").ap(),
        "g2": dt_("g2", [L, D], f32, kind="ExternalInput").ap(),
        "be2": dt_("be2", [L, D], f32, kind="ExternalInput").ap(),
        "out": dt_("out", [D, T], f32, kind="ExternalOutput").ap(),
    }
    if DEBUG_TAPS:
        for nm, shp in [("dbg_x", [D, T]), ("dbg_q", [P, T]), ("dbg_k", [P, T]),
                        ("dbg_v", [P, TT * DSH]), ("dbg_attn", [P, T]),
                        ("dbg_y1", [D, T]), ("dbg_x2", [D, T])]:
            io[nm] = dt_(nm, shp, f32, kind="ExternalOutput").ap()

    with tile.TileContext(nc) as tc:
        _build(tc, n_layers, io)
    nc.compile()
    return nc


def _build(tc, n_layers, io):
    from contextlib import ExitStack
    nc = tc.nc
    att_scale = 1.0 / np.sqrt(HD)

    # ------------------------------------------------ pools
    st = ExitStack()
    persist = st.enter_context(tc.tile_pool(name="persist", bufs=1))
    wpool = st.enter_context(tc.tile_pool(name="wpool", bufs=1))   # W1/W2
    wqkv = st.enter_context(tc.tile_pool(name="wqkv", bufs=1))     # Wq/Wk/Wv/Wo
    small = st.enter_context(tc.tile_pool(name="small", bufs=2))   # biases/stats
    tok8k = st.enter_context(tc.tile_pool(name="tok8k", bufs=2))   # [128, T]
    e512 = st.enter_context(tc.tile_pool(name="e512", bufs=6))     # [128, CH]
    htp = st.enter_context(tc.tile_pool(name="htp", bufs=2))       # [128,FMT,CH]
    ps = st.enter_context(tc.tile_pool(name="ps", bufs=5, space="PSUM"))
    pst = st.enter_context(tc.tile_pool(name="pst", bufs=2, space="PSUM"))
    drin = st.enter_context(tc.tile_pool(name="drin", bufs=2, space="DRAM"))
    drout = st.enter_context(tc.tile_pool(name="drout", bufs=2, space="DRAM"))

    # ------------------------------------------------ persistent tiles
    xbuf = persist.tile([P, DT, T], f32, name="xbuf")      # x / x2 (fp32)
    qT = persist.tile([P, T], f32, name="qT")              # Q^T shard
    kT = persist.tile([P, T], f32, name="kT")              # K^T shard
    vsb = persist.tile([P, TT, 2 * (HD + 1)], f32, name="vsb")  # [V|1|V|1]
    ident = persist.tile([P, P], f32, name="ident")
    eye8 = persist.tile([P, P], f32, name="eye8")
    onesP64 = persist.tile([P, 64], f32, name="onesP64")
    attnTA = persist.tile([HD, T], f32, name="attnTA")     # head-0 attn^T
    attnTB = persist.tile([HD, T], f32, name="attnTB")     # head-1 attn^T
    idxs = persist.tile([P, T // 16], i16, name="idxs")

    make_identity(nc, ident[:])
    nc.scalar.mul(_r(eye8[:]), ident[:], 1.0 / NC)         # (1/8) * I
    nc.vector.memset(onesP64[:], 1.0)
    nc.scalar.activation(_r(vsb[:, :, HD:HD + 1]), ident[:, 0:TT].unsqueeze(-1),
                         AF.Identity, bias=1.0, scale=0.0)
    nc.scalar.activation(_r(vsb[:, :, 2 * HD + 1:]), ident[:, 0:TT].unsqueeze(-1),
                         AF.Identity, bias=1.0, scale=0.0)
    # indices wrapped in 16 partitions, replicated into all 8 Q7-core stripes
    for r_ in range(P // 16):
        nc.sync.dma_start(idxs[16 * r_:16 * (r_ + 1), :], io["idx"])

    # ---------------------------------------- embedding: x^T = (emb[seq])^T + pes^T
    pes_lo = htp.tile([P, FMT, CH], f32, tag="ht", name="pes_lo")
    pes_hi = htp.tile([P, FMT, CH], f32, tag="ht", name="pes_hi")
    pes_r = io["pesT"].rearrange("(k p) s -> p k s", p=P)
    nc.sync.dma_start(pes_lo[:], pes_r[:, 0:4, :])
    nc.sync.dma_start(pes_hi[:], pes_r[:, 4:8, :])

    for half in range(TT // 2):  # gather 2 token-tiles (256 rows) at a time
        gtile = tok8k.tile([P, 2, D], f32, tag="tok", name=f"gt{half}")
        nc.gpsimd.dma_gather(
            out_ap=gtile[:],
            in_ap=io["emb"],
            idxs_ap=idxs[:, half * 16:(half + 1) * 16],
            num_idxs=2 * P,
            num_idxs_reg=2 * P,
            elem_size=D,
            queue_num=half % GATHER_QUEUES,
        )
        for j in range(2):
            t = half * 2 + j            # token tile index
            pos_t = t % (S // P)        # position tile within the batch
            for k in range(DT):
                ptile = pst.tile([P, P], f32, tag="tp", name=f"tp{t}_{k}")
                nc.tensor.transpose(ptile[:], gtile[:, j, ts(k, P)], ident[:])
                pes_src = pes_lo if k < 4 else pes_hi
                nc.vector.tensor_tensor(
                    out=_r(xbuf[:, k, ts(t, P)]),
                    in0=ptile[:],
                    in1=pes_src[:, k % 4, ts(pos_t, P)],
                    op=ALU.add,
                )

    if DEBUG_TAPS:
        nc.sync.dma_start(io["dbg_x"].rearrange("(k p) t -> p k t", p=P), xbuf[:])

    # ---------------------------------------- batchnorm (redundant, full-D)
    def batchnorm(lbl, arout_t, g_sb, be_sb):
        ysum = small.tile([P, DT], f32, tag="ysum", name=f"ysum{lbl}")
        sqp = small.tile([P, DT, NCH], f32, tag="sqp", name=f"sqp{lbl}")
        for k in range(DT):
            yt = tok8k.tile([P, T], f32, tag="tok", name=f"yt{lbl}_{k}")
            nc.sync.dma_start(yt[:], arout_t[ts(k, P), :])
            nc.vector.reduce_sum(out=ysum[:, k:k + 1], in_=yt[:],
                                 axis=mybir.AxisListType.X)
            for c in range(NCH):
                scr = e512.tile([P, CH], f32, tag="e", name=f"sq{lbl}_{k}_{c}")
                nc.scalar.activation(scr[:], yt[:, ts(c, CH)], AF.Square,
                                     accum_out=sqp[:, k, c:c + 1])
        sq = small.tile([P, DT], f32, tag="sq", name=f"sq{lbl}")
        nc.vector.reduce_sum(out=sq[:], in_=sqp[:], axis=mybir.AxisListType.X)
        mean = small.tile([P, DT], f32, tag="mean", name=f"mean{lbl}")
        nc.vector.tensor_scalar_mul(mean[:], ysum[:], 1.0 / T)
        msq = small.tile([P, DT], f32, tag="msq", name=f"msq{lbl}")
        nc.vector.tensor_tensor(out=msq[:], in0=mean[:], in1=mean[:], op=ALU.mult)
        veps = small.tile([P, DT], f32, tag="veps", name=f"veps{lbl}")
        # veps = sq/T - mean^2 + EPS
        nc.vector.scalar_tensor_tensor(out=veps[:], in0=sq[:], scalar=1.0 / T,
                                       in1=msq[:], op0=ALU.mult, op1=ALU.subtract)
        nc.vector.tensor_scalar_add(veps[:], veps[:], EPS)
        rec = small.tile([P, DT], f32, tag="rec", name=f"rec{lbl}")
        nc.vector.reciprocal(rec[:], veps[:])
        rstd = small.tile([P, DT], f32, tag="rstd", name=f"rstd{lbl}")
        nc.scalar.sqrt(rstd[:], rec[:])
        sc = small.tile([P, DT], f32, tag="sc", name=f"sc{lbl}")
        nc.vector.tensor_tensor(out=sc[:], in0=g_sb[:], in1=rstd[:], op=ALU.mult)
        sh = small.tile([P, DT], f32, tag="sh", name=f"sh{lbl}")
        nc.vector.tensor_tensor(out=sh[:], in0=mean[:], in1=sc[:], op=ALU.mult)
        nc.vector.tensor_tensor(out=sh[:], in0=be_sb[:], in1=sh[:], op=ALU.subtract)
        for k in range(DT):
            yt = tok8k.tile([P, T], f32, tag="tok", name=f"ya{lbl}_{k}")
            nc.sync.dma_start(yt[:], arout_t[ts(k, P), :])
            nc.scalar.activation(_r(xbuf[:, k, :]), yt[:], AF.Identity,
                                 bias=sh[:, k:k + 1], scale=sc[:, k:k + 1])

    # ---------------------------------------- layers
    for l in range(n_layers):
        # ---- layer weights/params to SBUF
        wq_sb = wqkv.tile([P, DT, DSH], f32, tag="wq", name=f"wq{l}")
        wk_sb = wqkv.tile([P, DT, DSH], f32, tag="wk", name=f"wk{l}")
        wv_sb = wqkv.tile([P, DT, DSH], f32, tag="wv", name=f"wv{l}")
        wo_sbA = wqkv.tile([HD, D], f32, tag="woA", name=f"woA{l}")
        wo_sbB = wqkv.tile([HD, D], f32, tag="woB", name=f"woB{l}")
        w1_sb = wpool.tile([P, DT, FSH], f32, tag="w1", name=f"w1{l}")
        w2_sb = wpool.tile([P, KL, D], f32, tag="w2", name=f"w2{l}")
        nc.sync.dma_start(_r(wq_sb[:]), _r(io["wq"][l].rearrange("(k p) m -> p k m", p=P)))
        nc.sync.dma_start(_r(wk_sb[:]), _r(io["wk"][l].rearrange("(k p) m -> p k m", p=P)))
        nc.sync.dma_start(_r(wv_sb[:]), _r(io["wv"][l].rearrange("(k p) m -> p k m", p=P)))
        nc.sync.dma_start(_r(wo_sbA[:]), _r(io["wo"][l][0:HD, :]))
        nc.sync.dma_start(_r(wo_sbB[:]), _r(io["wo"][l][HD:2 * HD, :]))
        nc.sync.dma_start(_r(w1_sb[:]), _r(io["w1"][l].rearrange("(k p) m -> p k m", p=P)))
        nc.sync.dma_start(_r(w2_sb[:]), _r(io["w2"][l].rearrange("(k p) m -> p k m", p=P)))

        bq_sb = small.tile([P, 1], f32, tag="bq", name=f"bq{l}")
        bk_sb = small.tile([P, 1], f32, tag="bk", name=f"bk{l}")
        bv_sb = small.tile([P, 1], f32, tag="bv", name=f"bv{l}")
        b1_sb = small.tile([P, FMT], f32, tag="b1", name=f"b1{l}")
        nc.sync.dma_start(bq_sb[:], io["bq"][l].rearrange("(p o) -> p o", o=1))
        nc.sync.dma_start(bk_sb[:], io["bk"][l].rearrange("(p o) -> p o", o=1))
        nc.sync.dma_start(bv_sb[:], io["bv"][l].rearrange("(p o) -> p o", o=1))
        nc.sync.dma_start(b1_sb[:], io["b1"][l].rearrange("(m p) -> p m", p=P))

        g1_sb = small.tile([P, DT], f32, tag="g1", name=f"g1{l}")
        be1_sb = small.tile([P, DT], f32, tag="be1", name=f"be1{l}")
        g2_sb = small.tile([P, DT], f32, tag="g2", name=f"g2{l}")
        be2_sb = small.tile([P, DT], f32, tag="be2", name=f"be2{l}")
        nc.sync.dma_start(g1_sb[:], io["g1"][l].rearrange("(k p) -> p k", p=P))
        nc.sync.dma_start(be1_sb[:], io["be1"][l].rearrange("(k p) -> p k", p=P))
        nc.sync.dma_start(g2_sb[:], io["g2"][l].rearrange("(k p) -> p k", p=P))
        nc.sync.dma_start(be2_sb[:], io["be2"][l].rearrange("(k p) -> p k", p=P))

        # ---- QKV projections (shard): Q^T/K^T/V^T = W_shard^T @ x^T
        vT = tok8k.tile([P, T], f32, tag="tok", name=f"vT{l}")
        for c in range(NCH):
            psq = ps.tile([P, CH], f32, tag="mm", name=f"psq{l}_{c}")
            psk = ps.tile([P, CH], f32, tag="mm", name=f"psk{l}_{c}")
            psv = ps.tile([P, CH], f32, tag="mm", name=f"psv{l}_{c}")
            for k in range(DT):
                fl, ll = (k == 0), (k == DT - 1)
                rhs = _r(xbuf[:, k, ts(c, CH)])
                nc.tensor.matmul(psq[:], _r(wq_sb[:, k, :]), rhs, start=fl, stop=ll)
                nc.tensor.matmul(psk[:], _r(wk_sb[:, k, :]), rhs, start=fl, stop=ll)
                nc.tensor.matmul(psv[:], _r(wv_sb[:, k, :]), rhs, start=fl, stop=ll)
            nc.scalar.activation(_r(qT[:, ts(c, CH)]), psq[:], AF.Identity, bias=bq_sb[:])
            nc.scalar.activation(_r(kT[:, ts(c, CH)]), psk[:], AF.Identity, bias=bk_sb[:])
            nc.scalar.activation(vT[:, ts(c, CH)], psv[:], AF.Identity, bias=bv_sb[:])

        # ---- V^T -> V (token-partition layout) via PE transposes
        for t in range(TT):
            ptile = pst.tile([P, P], f32, tag="tp", name=f"vt{l}_{t}")
            nc.tensor.transpose(ptile[:], vT[:, ts(t, P)], ident[:])
            nc.vector.tensor_copy(
                _r(vsb[:, t, :].rearrange("p (h x) -> p h x", h=2)[:, :, 0:HD]),
                ptile[:].rearrange("p (h x) -> p h x", h=2))

        # ---- attention: per head all-f32r at PSUM base 0; softmax sums
        # fused into the U matmul via the ones-column appended to V.
        for b in range(B):
            for h, attnT_h in enumerate([attnTA, attnTB]):
                hp = h * HD
                vof = h * (HD + 1)
                ets = []
                for sk in range(B):
                    pss = ps.tile([P, CH], f32, tag="mm",
                                  name=f"pss{l}_{b}_{h}_{sk}")
                    nc.tensor.matmul(
                        pss[:],
                        _r(kT[hp:hp + HD, b * CH + sk * P:b * CH + (sk + 1) * P]),
                        _r(qT[hp:hp + HD, ts(b, CH)]),
                        start=True, stop=True)
                    et = e512.tile([P, CH], f32, tag="e",
                                   name=f"et{l}_{b}_{h}_{sk}")
                    nc.scalar.activation(_r(et[:]), pss[:], AF.Exp, scale=att_scale)
                    ets.append(et)
                psu = ps.tile([P, CH], f32, tag="mm", name=f"psu{l}_{b}_{h}")
                for sk in range(B):
                    nc.tensor.matmul(psu[0:HD + 1, :],
                                     _r(vsb[:, b * 4 + sk, vof:vof + HD + 1]),
                                     _r(ets[sk][:]),
                                     start=(sk == 0), stop=(sk == B - 1))
                rsb = e512.tile([P, CH], f32, tag="e", name=f"rsb{l}_{b}_{h}")
                nc.vector.reciprocal(rsb[HD:HD + 1, :], psu[HD:HD + 1, :])
                psr = ps.tile([P, CH], f32, tag="mm", name=f"psr{l}_{b}_{h}")
                nc.tensor.matmul(psr[0:HD, :], onesP64[HD:HD + 1, :],
                                 rsb[HD:HD + 1, :], start=True, stop=True)
                usb = e512.tile([P, CH], f32, tag="e", name=f"usb{l}_{b}_{h}")
                nc.scalar.copy(usb[0:HD, :], psu[0:HD, :])
                nc.vector.tensor_tensor(out=_r(attnT_h[:, ts(b, CH)]),
                                        in0=usb[0:HD, :],
                                        in1=psr[0:HD, :], op=ALU.mult)

        if DEBUG_TAPS and l == 0:
            nc.sync.dma_start(io["dbg_q"], qT[:])
            nc.sync.dma_start(io["dbg_k"], kT[:])
            nc.sync.dma_start(io["dbg_v"], vsb[:].rearrange("p a b -> p (a b)"))
            nc.sync.dma_start(io["dbg_attn"], attnTA[:].rearrange("p t -> p t"))

        # ---- Wo partial + residual/8 -> AllReduce
        arin1 = drin.tile([D, T], f32, tag="ari", name=f"ari1_{l}")
        arout1 = drout.tile([D, T], f32, tag="aro", addr_space="Shared",
                            name=f"aro1_{l}")
        for m in range(DT):
            for c in range(NCH):
                ps2 = ps.tile([P, CH], f32, tag="mm", name=f"pso{l}_{m}_{c}")
                nc.tensor.matmul(ps2[:], _r(wo_sbA[:, ts(m, P)]),
                                 _r(attnTA[:, ts(c, CH)]), start=True, stop=False)
                nc.tensor.matmul(ps2[:], _r(wo_sbB[:, ts(m, P)]),
                                 _r(attnTB[:, ts(c, CH)]), start=False, stop=False)
                nc.tensor.matmul(ps2[:], _r(eye8[:]), _r(xbuf[:, m, ts(c, CH)]),
                                 start=False, stop=True)
                osb = e512.tile([P, CH], f32, tag="e", name=f"osb{l}_{m}_{c}")
                nc.vector.tensor_copy(osb[:], ps2[:])
                nc.sync.dma_start(arin1[ts(m, P), ts(c, CH)], osb[:])
        nc.gpsimd.collective_compute(
            "AllReduce", ALU.add, replica_groups=REPLICAS,
            ins=[arin1.opt()], outs=[arout1.opt()])

        if DEBUG_TAPS and l == 0:
            nc.sync.dma_start(io["dbg_y1"], arout1)

        # ---- BN1 -> x2 into xbuf
        batchnorm(f"a{l}", arout1, g1_sb, be1_sb)
        if DEBUG_TAPS and l == 0:
            nc.sync.dma_start(io["dbg_x2"].rearrange("(k p) t -> p k t", p=P),
                              xbuf[:])

        # ---- FFN (chunk-major so h^T is chunk-resident) + residual/8 -> AR
        arin2 = drin.tile([D, T], f32, tag="ari", name=f"ari2_{l}")
        arout2 = drout.tile([D, T], f32, tag="aro", addr_space="Shared",
                            name=f"aro2_{l}")
        for c in range(NCH):
            ht = htp.tile([P, FMT, CH], f32, tag="ht", name=f"ht{l}_{c}")
            for m in range(FMT):
                ps1 = ps.tile([P, CH], f32, tag="mm", name=f"ps1{l}_{c}_{m}")
                for k in range(DT):
                    nc.tensor.matmul(ps1[:], _r(w1_sb[:, k, ts(m, P)]),
                                     _r(xbuf[:, k, ts(c, CH)]),
                                     start=(k == 0), stop=(k == DT - 1))
                nc.scalar.activation(_r(ht[:, m, :]), ps1[:], AF.Relu,
                                     bias=b1_sb[:, m:m + 1])
            for m in range(DT):
                ps2 = ps.tile([P, CH], f32, tag="mm", name=f"ps2{l}_{c}_{m}")
                for k in range(KL):
                    nc.tensor.matmul(ps2[:], _r(w2_sb[:, k, ts(m, P)]),
                                     _r(ht[:, k, :]), start=(k == 0), stop=False)
                nc.tensor.matmul(ps2[:], _r(eye8[:]), _r(xbuf[:, m, ts(c, CH)]),
                                 start=False, stop=True)
                osb = e512.tile([P, CH], f32, tag="e", name=f"fsb{l}_{c}_{m}")
                nc.vector.tensor_copy(osb[:], ps2[:])
                nc.sync.dma_start(arin2[ts(m, P), ts(c, CH)], osb[:])
        nc.gpsimd.collective_compute(
            "AllReduce", ALU.add, replica_groups=REPLICAS,
            ins=[arin2.opt()], outs=[arout2.opt()])

        # ---- BN2 -> x(l+1) into xbuf
        batchnorm(f"f{l}", arout2, g2_sb, be2_sb)

    # ---------------------------------------- output x^T -> [D, T]
    nc.sync.dma_start(io["out"].rearrange("(k p) t -> p k t", p=P), xbuf[:])
    st.close()


# ================================================================ host side

def make_in_maps(inputs):
    f = lambda a: np.ascontiguousarray(np.asarray(a), dtype=np.float32)
    seq = np.asarray(inputs["sequence"]).reshape(-1).astype(np.int16)
    idx = np.ascontiguousarray(seq.reshape(T // 16, 16).T)     # [16, T//16]
    emb = f(inputs["emb"])
    pesT = np.ascontiguousarray(f(inputs["pes"]).T)            # [D, S]
    Wq, Wk, Wv = f(inputs["Wq"]), f(inputs["Wk"]), f(inputs["Wv"])
    Wo, W1, W2 = f(inputs["Wo"]), f(inputs["W1"]), f(inputs["W2"])
    bq, bk, bv = f(inputs["bq"]), f(inputs["bk"]), f(inputs["bv"])
    b1 = f(inputs["b1"])
    g1, be1 = f(inputs["g1"]), f(inputs["be1"])
    g2, be2 = f(inputs["g2"]), f(inputs["be2"])

    in_maps = []
    for c in range(NC):
        ds_ = slice(c * DSH, (c + 1) * DSH)
        fs_ = slice(c * FSH, (c + 1) * FSH)
        in_maps.append({
            "emb": emb,
            "idx": idx,
            "pesT": pesT,
            "wq": np.ascontiguousarray(Wq[:, :, ds_]),
            "wk": np.ascontiguousarray(Wk[:, :, ds_]),
            "wv": np.ascontiguousarray(Wv[:, :, ds_]),
            "wo": np.ascontiguousarray(Wo[:, ds_, :]),
            "w1": np.ascontiguousarray(W1[:, :, fs_]),
            "w2": np.ascontiguousarray(W2[:, fs_, :]),
            "bq": np.ascontiguousarray(bq[:, ds_]),
            "bk": np.ascontiguousarray(bk[:, ds_]),
            "bv": np.ascontiguousarray(bv[:, ds_]),
            "b1": np.ascontiguousarray(b1[:, fs_]),
            "g1": g1, "be1": be1, "g2": g2, "be2": be2,
        })
    return in_maps


_CACHE = {}


def _get_module():
    if "nc" not in _CACHE:
        _CACHE["nc"] = build_module()
    return _CACHE["nc"]


def kernel(**inputs):
    from concourse import bass_utils
    nc = _get_module()
    in_maps = make_in_maps(inputs)
    res = bass_utils.run_bass_kernel_spmd(nc, in_maps, list(range(NC)))
    o = np.asarray(res.results[0]["out"])                  # [D, T]
    return np.ascontiguousarray(o.T).reshape(B, S, D).astype(np.float32)



# revision 12
# speedup vs baseline: 2.4074x; 2.4074x over previous
"""Trainium2 Bass kernel for a 6-layer post-BatchNorm transformer encoder.

Reference model:
  x = emb[seq] + pes                                  # [B,S,D] = [4,512,1024]
  6x: x = BN(x + attn(x)); x = BN(x + ffn(x))
  BN = per-channel batch stats over (B,S), eps=1e-3.

Sharding: data-parallel over tokens across 8 NeuronCores. Core c owns the
256 tokens  [batch c//2, sequence half c%2].  Weights are replicated
(streamed from HBM in bf16, host-converted).  Per layer the only
communication is:
  - a pair AllGather ([[0,1],[2,3],...]) exchanging K^T and token-major V
    (1MB bf16) so each core holds its batch's full 512-key sequence, and
  - two 8KB 8-rank AllReduces for the BatchNorm batch statistics
    (sum / sum-of-squares per channel).
This removes the TP-style [D,T] activation AllReduces entirely.

Numerics: matmul operands bf16 (weights + activation mirrors), PSUM
accumulation fp32, residual/BN arithmetic fp32.  x master kept fp32.
Attention per (head): scores^T = K_h @ Q_h^T (K=64 contraction),
E = exp(scale*scores^T) in bf16 (max-subtraction skipped; scores are O(1)),
U^T = V_h^T @ E^T with denominators from a ones-column appended to V,
normalized by a PE-broadcast reciprocal row.  bo/b2 biases cancel inside
BN and are dropped.  Host does input marshalling only: embedding lookup
(emb[seq]+pes), weight bf16 conversion + chunk-major relayout, output
reassembly.
"""

import os

import numpy as np

import concourse.bass as bass
import concourse.mybir as mybir
import concourse.tile as tile
from concourse import bacc
from concourse.bass import ts
from concourse.masks import make_identity

# ---------------------------------------------------------------- dims
V, D, L, H, B, S = 32000, 1024, 6, 16, 4, 512
HD = D // H            # 64
DF = 4 * D             # 4096
EPS = 1e-3
NC = 8                 # cores
T = B * S              # 2048 tokens (global, for BN stats)
P = 128                # partitions
TL = 256               # local tokens per core
DT = D // P            # 8 d-tiles
FT = DF // P           # 32 ffn hidden tiles
KT = S // P            # 4 key tiles (full sequence)

f32 = mybir.dt.float32
bf16 = mybir.dt.bfloat16
f32r = mybir.dt.float32r
AF = mybir.ActivationFunctionType
ALU = mybir.AluOpType

PAIRS = [[2 * i, 2 * i + 1] for i in range(4)]
ALL8 = [list(range(NC))]

N_LAYERS = int(os.environ.get("TRN_KERNEL_LAYERS", str(L)))
DEBUG_TAPS = os.environ.get("TRN_KERNEL_DEBUG", "0") == "1"


def _r(ap):
    """view an fp32 AP as float32r for full-rate PE matmul"""
    return ap.bitcast(f32r)


def build_module(n_layers=None):
    if n_layers is None:
        n_layers = N_LAYERS
    nc = bacc.Bacc("TRN2", target_bir_lowering=False, debug=False,
                   num_devices=NC)

    dt_ = nc.dram_tensor
    io = {
        "x0": dt_("x0", [P, DT, TL], f32, kind="ExternalInput").ap(),
        # weight chunks, host-prelaid so every chunk DMA is contiguous 1MB
        "wq": dt_("wq", [L, 2, P, DT, 512], bf16, kind="ExternalInput").ap(),
        "wk": dt_("wk", [L, 2, P, DT, 512], bf16, kind="ExternalInput").ap(),
        "wv": dt_("wv", [L, 2, P, DT, 512], bf16, kind="ExternalInput").ap(),
        "wo": dt_("wo", [L, 2, P, DT, 512], bf16, kind="ExternalInput").ap(),
        "w1": dt_("w1", [L, 8, P, DT, 512], bf16, kind="ExternalInput").ap(),
        "w2": dt_("w2", [L, 8, P, FT, P], bf16, kind="ExternalInput").ap(),
        "bq": dt_("bq", [L, P, DT], f32, kind="ExternalInput").ap(),
        "bk": dt_("bk", [L, P, DT], f32, kind="ExternalInput").ap(),
        "bv": dt_("bv", [L, P, DT], f32, kind="ExternalInput").ap(),
        "b1": dt_("b1", [L, P, FT], f32, kind="ExternalInput").ap(),
        "g1": dt_("g1", [L, P, DT], f32, kind="ExternalInput").ap(),
        "be1": dt_("be1", [L, P, DT], f32, kind="ExternalInput").ap(),
        "g2": dt_("g2", [L, P, DT], f32, kind="ExternalInput").ap(),
        "be2": dt_("be2", [L, P, DT], f32, kind="ExternalInput").ap(),
        "out": dt_("out", [D, TL], f32, kind="ExternalOutput").ap(),
    }
    if DEBUG_TAPS:
        for nm, shp, dt in [("dbg_q", [P, DT, TL], bf16),
                            ("dbg_k", [P, DT, S], bf16),
                            ("dbg_v", [P, KT, H, HD + 1], bf16),
                            ("dbg_attn", [P, DT, TL], bf16),
                            ("dbg_y1", [P, DT, TL], f32),
                            ("dbg_x2", [P, DT, TL], f32)]:
            io[nm] = dt_(nm, shp, dt, kind="ExternalOutput").ap()

    with tile.TileContext(nc) as tc:
        _build(tc, n_layers, io)
    nc.compile()
    return nc


def _build(tc, n_layers, io):
    from contextlib import ExitStack
    nc = tc.nc
    att_scale = 1.0 / np.sqrt(HD)

    # ------------------------------------------------ pools
    st = ExitStack()
    persist = st.enter_context(tc.tile_pool(name="persist", bufs=1))
    wc8 = st.enter_context(tc.tile_pool(name="wc8", bufs=5))    # [P,8,512] bf16
    wc32 = st.enter_context(tc.tile_pool(name="wc32", bufs=3))  # [P,32,128] bf16
    small = st.enter_context(tc.tile_pool(name="small", bufs=2))
    epool = st.enter_context(tc.tile_pool(name="epool", bufs=8))
    ps = st.enter_context(tc.tile_pool(name="ps", bufs=4, space="PSUM"))
    pacc = st.enter_context(tc.tile_pool(name="pacc", bufs=2, space="PSUM"))
    pst = st.enter_context(tc.tile_pool(name="pst", bufs=2, space="PSUM"))
    drin = st.enter_context(tc.tile_pool(name="drin", bufs=2, space="DRAM"))
    drout = st.enter_context(tc.tile_pool(name="drout", bufs=2, space="DRAM"))

    # ------------------------------------------------ persistent tiles
    x = persist.tile([P, DT, TL], f32, name="x")          # x master
    xb = persist.tile([P, DT, TL], bf16, name="xb")       # bf16 mirror
    y = persist.tile([P, DT, TL], f32, name="y")          # x + sublayer(x)
    qT = persist.tile([P, DT, TL], bf16, name="qT")
    kloc = persist.tile([P, DT, TL], bf16, name="kloc")   # local K^T
    vT = persist.tile([P, DT, TL], bf16, name="vT")       # local V^T
    vloc = persist.tile([P, 2, H, HD], bf16, name="vloc")  # local V tok-major
    kT = persist.tile([P, DT, S], bf16, name="kT")        # full K^T
    vsb = persist.tile([P, KT, H, HD + 1], bf16, name="vsb")  # V | ones
    attnT = persist.tile([P, DT, TL], bf16, name="attnT")
    attnTB = persist.tile([64, DT, TL], bf16, name="attnTB")  # odd heads
    h = persist.tile([P, FT, TL], bf16, name="h")         # ffn hidden
    identb = persist.tile([P, P], bf16, name="identb")
    onesP = persist.tile([P, 64], f32, name="onesP")

    make_identity(nc, identb[:])
    nc.vector.memset(onesP[:], 1.0)
    nc.vector.memset(vsb[:, :, :, HD:HD + 1], 1.0)        # ones lane

    # ---------------------------------------- x = x0 (host: emb[seq]+pes)
    nc.sync.dma_start(x[:], io["x0"])
    for k in range(DT):
        nc.vector.tensor_copy(xb[:, k, :], x[:, k, :])

    # ---------------------------------------- batchnorm helper
    def batchnorm(lbl, g_sb, be_sb):
        """y -> x (fp32) and xb (bf16), exact global stats via 8KB AR."""
        stt = small.tile([P, 16], f32, tag="stt", name=f"stt{lbl}")
        for k in range(DT):
            nc.vector.reduce_sum(out=stt[:, k:k + 1], in_=y[:, k, :],
                                 axis=mybir.AxisListType.X)
            scr = epool.tile([P, TL], f32, tag="e", name=f"sq{lbl}_{k}")
            nc.scalar.activation(scr[:], y[:, k, :], AF.Square,
                                 accum_out=stt[:, 8 + k:9 + k])
        arin = drin.tile([P, 16], f32, tag="ari", name=f"ari{lbl}")
        arout = drout.tile([P, 16], f32, tag="aro", addr_space="Shared",
                           name=f"aro{lbl}")
        nc.gpsimd.dma_start(arin[:], stt[:])
        nc.gpsimd.collective_compute(
            "AllReduce", ALU.add, replica_groups=ALL8,
            ins=[arin.opt()], outs=[arout.opt()])
        ast = small.tile([P, 16], f32, tag="ast", name=f"ast{lbl}")
        nc.gpsimd.dma_start(ast[:], arout[:])
        mean = small.tile([P, DT], f32, tag="mean", name=f"mean{lbl}")
        nc.vector.tensor_scalar_mul(mean[:], ast[:, 0:8], 1.0 / T)
        msq = small.tile([P, DT], f32, tag="msq", name=f"msq{lbl}")
        nc.vector.tensor_tensor(out=msq[:], in0=mean[:], in1=mean[:],
                                op=ALU.mult)
        veps = small.tile([P, DT], f32, tag="veps", name=f"veps{lbl}")
        nc.vector.scalar_tensor_tensor(out=veps[:], in0=ast[:, 8:16],
                                       scalar=1.0 / T, in1=msq[:],
                                       op0=ALU.mult, op1=ALU.subtract)
        nc.vector.tensor_scalar_add(veps[:], veps[:], EPS)
        rec = small.tile([P, DT], f32, tag="rec", name=f"rec{lbl}")
        nc.vector.reciprocal(rec[:], veps[:])
        rstd = small.tile([P, DT], f32, tag="rstd", name=f"rstd{lbl}")
        nc.scalar.sqrt(rstd[:], rec[:])
        sc = small.tile([P, DT], f32, tag="sc", name=f"sc{lbl}")
        nc.vector.tensor_tensor(out=sc[:], in0=g_sb[:], in1=rstd[:],
                                op=ALU.mult)
        sh = small.tile([P, DT], f32, tag="sh", name=f"sh{lbl}")
        nc.vector.tensor_tensor(out=sh[:], in0=mean[:], in1=sc[:], op=ALU.mult)
        nc.vector.tensor_tensor(out=sh[:], in0=be_sb[:], in1=sh[:],
                                op=ALU.subtract)
        for k in range(DT):
            nc.scalar.activation(x[:, k, :], y[:, k, :], AF.Identity,
                                 bias=sh[:, k:k + 1], scale=sc[:, k:k + 1])
            nc.vector.tensor_copy(xb[:, k, :], x[:, k, :])

    # ---------------------------------------- layers
    for l in range(n_layers):
        # ---- per-layer small params
        bq_sb = small.tile([P, DT], f32, tag="bq", name=f"bq{l}")
        bk_sb = small.tile([P, DT], f32, tag="bk", name=f"bk{l}")
        bv_sb = small.tile([P, DT], f32, tag="bv", name=f"bv{l}")
        b1_sb = small.tile([P, FT], f32, tag="b1", name=f"b1{l}")
        g1_sb = small.tile([P, DT], f32, tag="g1", name=f"g1{l}")
        be1_sb = small.tile([P, DT], f32, tag="be1", name=f"be1{l}")
        g2_sb = small.tile([P, DT], f32, tag="g2", name=f"g2{l}")
        be2_sb = small.tile([P, DT], f32, tag="be2", name=f"be2{l}")
        for nm, t_ in [("bq", bq_sb), ("bk", bk_sb), ("bv", bv_sb),
                       ("b1", b1_sb), ("g1", g1_sb), ("be1", be1_sb),
                       ("g2", g2_sb), ("be2", be2_sb)]:
            nc.sync.dma_start(t_[:], io[nm][l])

        # ---- QKV projections (local tokens, all heads)
        for half in range(2):
            wqc = wc8.tile([P, DT, 512], bf16, tag="wc", name=f"wq{l}_{half}")
            wkc = wc8.tile([P, DT, 512], bf16, tag="wc", name=f"wk{l}_{half}")
            wvc = wc8.tile([P, DT, 512], bf16, tag="wc", name=f"wv{l}_{half}")
            nc.sync.dma_start(wqc[:], io["wq"][l, half])
            nc.sync.dma_start(wkc[:], io["wk"][l, half])
            nc.sync.dma_start(wvc[:], io["wv"][l, half])
            for wi, (wcb, dst, bias) in enumerate([(wqc, qT, bq_sb),
                                                   (wkc, kloc, bk_sb),
                                                   (wvc, vT, bv_sb)]):
                for m in range(4):
                    mt = half * 4 + m
                    psq = ps.tile([P, TL], f32, tag="mm",
                                  name=f"q{l}_{half}_{wi}_{m}")
                    for k in range(DT):
                        nc.tensor.matmul(psq[:], wcb[:, k, ts(m, P)],
                                         xb[:, k, :],
                                         start=(k == 0), stop=(k == DT - 1))
                    nc.scalar.activation(dst[:, mt, :], psq[:], AF.Identity,
                                         bias=bias[:, mt:mt + 1])

        # ---- local V^T -> token-major V
        for tt in range(2):
            for k in range(DT):
                ptile = pst.tile([P, P], bf16, tag="tp", name=f"vt{l}_{tt}_{k}")
                nc.tensor.transpose(ptile[:], vT[:, k, ts(tt, P)], identb[:])
                nc.vector.tensor_copy(
                    vloc[:, tt, 2 * k:2 * k + 2, :],
                    ptile[:].rearrange("p (h e) -> p h e", h=2))

        # ---- pair AllGather of [K^T | V]
        agin = drin.tile([P, 2 * DT * TL], bf16, tag="agi", name=f"agi{l}")
        ago = drout.tile([2, P, 2 * DT * TL], bf16, tag="ago", name=f"ago{l}")
        nc.gpsimd.dma_start(
            agin[:, 0:DT * TL].rearrange("p (k t) -> p k t", k=DT), kloc[:])
        nc.gpsimd.dma_start(
            agin[:, DT * TL:].rearrange("p (a h e) -> p a h e", a=2, h=H),
            vloc[:])
        nc.gpsimd.collective_compute(
            "AllGather", ALU.bypass, replica_groups=PAIRS,
            ins=[agin.opt()], outs=[ago.opt()])
        for r in range(2):
            nc.gpsimd.dma_start(
                kT[:, :, r * TL:(r + 1) * TL],
                ago[r, :, 0:DT * TL].rearrange("p (k t) -> p k t", k=DT))
            nc.gpsimd.dma_start(
                vsb[:, 2 * r:2 * r + 2, :, 0:HD],
                ago[r, :, DT * TL:].rearrange("p (a h e) -> p a h e",
                                              a=2, h=H))

        # ---- attention per head
        for hd_ in range(H):
            hb = (hd_ % 2) * HD                 # partition base within d-tile
            kk = hd_ // 2                       # d-tile
            ets = []
            for kt in range(KT):
                pss = ps.tile([P, TL], f32, tag="mm", name=f"ps{l}_{hd_}_{kt}")
                nc.tensor.matmul(pss[:], kT[hb:hb + HD, kk, ts(kt, P)],
                                 qT[hb:hb + HD, kk, :], start=True, stop=True)
                et = epool.tile([P, TL], bf16, tag="eb", name=f"et{l}_{hd_}_{kt}")
                nc.scalar.activation(et[:], pss[:], AF.Exp, scale=att_scale)
                ets.append(et)
            psu = ps.tile([P, TL], f32, tag="mm", name=f"pu{l}_{hd_}")
            for kt in range(KT):
                nc.tensor.matmul(psu[0:HD + 1, :], vsb[:, kt, hd_, :],
                                 ets[kt][:], start=(kt == 0),
                                 stop=(kt == KT - 1))
            rsb = epool.tile([P, TL], f32r, tag="er", name=f"rs{l}_{hd_}")
            with nc.allow_low_precision(reason="f32r == f32 precision"):
                nc.vector.reciprocal(rsb[HD:HD + 1, :], psu[HD:HD + 1, :])
            psr = ps.tile([P, TL], f32, tag="mm", name=f"pr{l}_{hd_}")
            nc.tensor.matmul(psr[0:HD, :], _r(onesP[HD:HD + 1, :]),
                             rsb[HD:HD + 1, :], start=True, stop=True)
            usb = epool.tile([P, TL], f32, tag="e", name=f"us{l}_{hd_}")
            nc.scalar.copy(usb[0:HD, :], psu[0:HD, :])
            dst = (attnT[0:HD, kk, :] if hd_ % 2 == 0 else attnTB[:, kk, :])
            nc.vector.tensor_tensor(out=dst, in0=usb[0:HD, :],
                                    in1=psr[0:HD, :], op=ALU.mult)
        nc.sync.dma_start(attnT[HD:P, :, :], attnTB[:])

        # ---- Wo + residual -> y
        for half in range(2):
            woc = wc8.tile([P, DT, 512], bf16, tag="wc", name=f"wo{l}_{half}")
            nc.sync.dma_start(woc[:], io["wo"][l, half])
            for m in range(4):
                mt = half * 4 + m
                ps2 = ps.tile([P, TL], f32, tag="mm", name=f"o{l}_{half}_{m}")
                for k in range(DT):
                    nc.tensor.matmul(ps2[:], woc[:, k, ts(m, P)],
                                     attnT[:, k, :],
                                     start=(k == 0), stop=(k == DT - 1))
                nc.vector.tensor_tensor(out=y[:, mt, :], in0=ps2[:],
                                        in1=x[:, mt, :], op=ALU.add)

        if DEBUG_TAPS and l == 0:
            nc.sync.dma_start(io["dbg_q"], qT[:])
            nc.sync.dma_start(io["dbg_k"], kT[:])
            nc.sync.dma_start(io["dbg_v"], vsb[:])
            nc.sync.dma_start(io["dbg_attn"], attnT[:])
            nc.sync.dma_start(io["dbg_y1"], y[:])

        # ---- BN1 -> x, xb
        batchnorm(f"a{l}", g1_sb, be1_sb)
        if DEBUG_TAPS and l == 0:
            nc.sync.dma_start(io["dbg_x2"], x[:])

        # ---- FFN1: h = relu(W1^T x + b1)
        for c in range(8):
            w1c = wc8.tile([P, DT, 512], bf16, tag="wc", name=f"w1{l}_{c}")
            nc.sync.dma_start(w1c[:], io["w1"][l, c])
            for m in range(4):
                mt = c * 4 + m
                ps1 = ps.tile([P, TL], f32, tag="mm", name=f"f{l}_{c}_{m}")
                for k in range(DT):
                    nc.tensor.matmul(ps1[:], w1c[:, k, ts(m, P)], xb[:, k, :],
                                     start=(k == 0), stop=(k == DT - 1))
                nc.scalar.activation(h[:, mt, :], ps1[:], AF.Relu,
                                     bias=b1_sb[:, mt:mt + 1])

        # ---- FFN2 + residual -> y
        for m in range(DT):
            w2c = wc32.tile([P, FT, P], bf16, tag="wc2", name=f"w2{l}_{m}")
            nc.sync.dma_start(w2c[:], io["w2"][l, m])
            ps2 = pacc.tile([P, TL], f32, tag="acc", name=f"g{l}_{m}")
            for k in range(FT):
                nc.tensor.matmul(ps2[:], w2c[:, k, :], h[:, k, :],
                                 start=(k == 0), stop=(k == FT - 1))
            nc.vector.tensor_tensor(out=y[:, m, :], in0=ps2[:],
                                    in1=x[:, m, :], op=ALU.add)

        # ---- BN2 -> x, xb
        batchnorm(f"f{l}", g2_sb, be2_sb)

    # ---------------------------------------- output x -> [D, TL]
    nc.sync.dma_start(io["out"].rearrange("(k p) t -> p k t", p=P), x[:])
    st.close()


# ================================================================ host side

def make_in_maps(inputs):
    import ml_dtypes
    f = lambda a: np.ascontiguousarray(np.asarray(a), dtype=np.float32)
    b = lambda a: np.ascontiguousarray(np.asarray(a, dtype=np.float32)
                                       .astype(ml_dtypes.bfloat16))
    seq = np.asarray(inputs["sequence"]).astype(np.int64)       # [B, S]
    emb = f(inputs["emb"])
    pes = f(inputs["pes"])
    x0 = emb[seq] + pes[None, :, :]                             # [B, S, D]

    Wq, Wk, Wv = inputs["Wq"], inputs["Wk"], inputs["Wv"]
    Wo, W1, W2 = inputs["Wo"], inputs["W1"], inputs["W2"]

    def chunk8(W, c):
        # [L, D, M] -> [L, c, P, DT, M//c]  with lhsT layout [k*P+p, m]
        W = np.asarray(W, dtype=np.float32)
        Lw, Dw, M = W.shape
        W = W.reshape(Lw, DT, P, c, M // c)
        return np.ascontiguousarray(
            W.transpose(0, 3, 2, 1, 4)).astype(ml_dtypes.bfloat16)

    def chunk_w2(W):
        # [L, DF, D] -> [L, 8, P, FT, P]: chunk m-tiles, k full
        W = np.asarray(W, dtype=np.float32)
        W = W.reshape(L, FT, P, DT, P)
        return np.ascontiguousarray(
            W.transpose(0, 3, 2, 1, 4)).astype(ml_dtypes.bfloat16)

    def vecP(v, n):
        # [L, n*P] -> [L, P, n]
        v = np.asarray(v, dtype=np.float32).reshape(L, n, P)
        return np.ascontiguousarray(v.transpose(0, 2, 1))

    shared = {
        "wq": chunk8(Wq, 2), "wk": chunk8(Wk, 2), "wv": chunk8(Wv, 2),
        "wo": chunk8(Wo, 2), "w1": chunk8(W1, 8), "w2": chunk_w2(W2),
        "bq": vecP(inputs["bq"], DT), "bk": vecP(inputs["bk"], DT),
        "bv": vecP(inputs["bv"], DT), "b1": vecP(inputs["b1"], FT),
        "g1": vecP(inputs["g1"], DT), "be1": vecP(inputs["be1"], DT),
        "g2": vecP(inputs["g2"], DT), "be2": vecP(inputs["be2"], DT),
    }

    in_maps = []
    for c in range(NC):
        bi, hf = c // 2, c % 2
        xs = x0[bi, hf * TL:(hf + 1) * TL, :]                   # [TL, D]
        x0T = np.ascontiguousarray(
            xs.T.reshape(DT, P, TL).transpose(1, 0, 2))          # [P, DT, TL]
        m = {"x0": x0T}
        m.update(shared)
        in_maps.append(m)
    return in_maps


def assemble_output(res):
    out = np.empty((B, S, D), dtype=np.float32)
    for c in range(NC):
        o = np.asarray(res.results[c]["out"])                   # [D, TL]
        bi, hf = c // 2, c % 2
        out[bi, hf * TL:(hf + 1) * TL, :] = o.T
    return out


_CACHE = {}


def _get_module():
    if "nc" not in _CACHE:
        _CACHE["nc"] = build_module()
    return _CACHE["nc"]


def kernel(**inputs):
    from concourse import bass_utils
    nc = _get_module()
    in_maps = make_in_maps(inputs)
    res = bass_utils.run_bass_kernel_spmd(nc, in_maps, list(range(NC)))
    return assemble_output(res)


# revision 17
# speedup vs baseline: 2.4162x; 1.0037x over previous
"""Trainium2 Bass kernel for a 6-layer post-BatchNorm transformer encoder.

Reference model:
  x = emb[seq] + pes                                  # [B,S,D] = [4,512,1024]
  6x: x = BN(x + attn(x)); x = BN(x + ffn(x))
  BN = per-channel batch stats over (B,S), eps=1e-3.

Sharding: data-parallel over tokens across 8 NeuronCores. Core c owns the
256 tokens  [batch c//2, sequence half c%2].  Weights are replicated
(streamed from HBM in bf16, host-converted).  Per layer the only
communication is:
  - a pair AllGather ([[0,1],[2,3],...]) exchanging K^T and token-major V
    (1MB bf16) so each core holds its batch's full 512-key sequence, and
  - two 8KB 8-rank AllReduces for the BatchNorm batch statistics
    (sum / sum-of-squares per channel).
This removes the TP-style [D,T] activation AllReduces entirely.

Numerics: matmul operands bf16 (weights + activation mirrors), PSUM
accumulation fp32, residual/BN arithmetic fp32.  x master kept fp32.
Attention per (head): scores^T = K_h @ Q_h^T (K=64 contraction),
E = exp(scale*scores^T) in bf16 (max-subtraction skipped; scores are O(1)),
U^T = V_h^T @ E^T with denominators from a ones-column appended to V,
normalized by a PE-broadcast reciprocal row.  bo/b2 biases cancel inside
BN and are dropped.  Host does input marshalling only: embedding lookup
(emb[seq]+pes), weight bf16 conversion + chunk-major relayout, output
reassembly.
"""

import os

import numpy as np

import concourse.bass as bass
import concourse.mybir as mybir
import concourse.tile as tile
from concourse import bacc
from concourse.bass import ts
from concourse.masks import make_identity

# ---------------------------------------------------------------- dims
V, D, L, H, B, S = 32000, 1024, 6, 16, 4, 512
HD = D // H            # 64
DF = 4 * D             # 4096
EPS = 1e-3
NC = 8                 # cores
T = B * S              # 2048 tokens (global, for BN stats)
P = 128                # partitions
TL = 256               # local tokens per core
DT = D // P            # 8 d-tiles
FT = DF // P           # 32 ffn hidden tiles
KT = S // P            # 4 key tiles (full sequence)

f32 = mybir.dt.float32
bf16 = mybir.dt.bfloat16
f32r = mybir.dt.float32r
AF = mybir.ActivationFunctionType
ALU = mybir.AluOpType

PAIRS = [[2 * i, 2 * i + 1] for i in range(4)]
ALL8 = [list(range(NC))]

N_LAYERS = int(os.environ.get("TRN_KERNEL_LAYERS", str(L)))
DEBUG_TAPS = os.environ.get("TRN_KERNEL_DEBUG", "0") == "1"


def _r(ap):
    """view an fp32 AP as float32r for full-rate PE matmul"""
    return ap.bitcast(f32r)


def build_module(n_layers=None):
    if n_layers is None:
        n_layers = N_LAYERS
    nc = bacc.Bacc("TRN2", target_bir_lowering=False, debug=False,
                   num_devices=NC)

    dt_ = nc.dram_tensor
    io = {
        "x0": dt_("x0", [P, DT, TL], f32, kind="ExternalInput").ap(),
        # weight chunks, host-prelaid so every chunk DMA is contiguous 1MB
        "wq": dt_("wq", [L, 2, P, DT, 512], bf16, kind="ExternalInput").ap(),
        "wk": dt_("wk", [L, 2, P, DT, 512], bf16, kind="ExternalInput").ap(),
        "wv": dt_("wv", [L, 2, P, DT, 512], bf16, kind="ExternalInput").ap(),
        "wo": dt_("wo", [L, 2, P, DT, 512], bf16, kind="ExternalInput").ap(),
        "w1": dt_("w1", [L, 8, P, DT, 512], bf16, kind="ExternalInput").ap(),
        "w2": dt_("w2", [L, 8, P, FT, P], bf16, kind="ExternalInput").ap(),
        "bq": dt_("bq", [L, P, DT], f32, kind="ExternalInput").ap(),
        "bk": dt_("bk", [L, P, DT], f32, kind="ExternalInput").ap(),
        "bv": dt_("bv", [L, P, DT], f32, kind="ExternalInput").ap(),
        "b1": dt_("b1", [L, P, FT], f32, kind="ExternalInput").ap(),
        "g1": dt_("g1", [L, P, DT], f32, kind="ExternalInput").ap(),
        "be1": dt_("be1", [L, P, DT], f32, kind="ExternalInput").ap(),
        "g2": dt_("g2", [L, P, DT], f32, kind="ExternalInput").ap(),
        "be2": dt_("be2", [L, P, DT], f32, kind="ExternalInput").ap(),
        "out": dt_("out", [D, TL], f32, kind="ExternalOutput").ap(),
    }
    if DEBUG_TAPS:
        for nm, shp, dt in [("dbg_q", [P, DT, TL], bf16),
                            ("dbg_k", [P, DT, S], bf16),
                            ("dbg_v", [P, KT, H, HD + 1], bf16),
                            ("dbg_attn", [P, DT, TL], bf16),
                            ("dbg_y1", [P, DT, TL], f32),
                            ("dbg_x2", [P, DT, TL], f32)]:
            io[nm] = dt_(nm, shp, dt, kind="ExternalOutput").ap()

    with tile.TileContext(nc) as tc:
        _build(tc, n_layers, io)
    nc.compile()
    return nc


def _build(tc, n_layers, io):
    from contextlib import ExitStack
    nc = tc.nc
    att_scale = 1.0 / np.sqrt(HD)

    # ------------------------------------------------ pools
    st = ExitStack()
    persist = st.enter_context(tc.tile_pool(name="persist", bufs=1))
    wc8 = st.enter_context(tc.tile_pool(name="wc8", bufs=5))    # [P,8,512] bf16
    wc32 = st.enter_context(tc.tile_pool(name="wc32", bufs=3))  # [P,32,128] bf16
    small = st.enter_context(tc.tile_pool(name="small", bufs=2))
    epool = st.enter_context(tc.tile_pool(name="epool", bufs=8))
    ps = st.enter_context(tc.tile_pool(name="ps", bufs=4, space="PSUM"))
    pu = st.enter_context(tc.tile_pool(name="pu", bufs=3, space="PSUM"))
    pst = st.enter_context(tc.tile_pool(name="pst", bufs=1, space="PSUM"))
    drin = st.enter_context(tc.tile_pool(name="drin", bufs=2, space="DRAM"))
    drout = st.enter_context(tc.tile_pool(name="drout", bufs=2, space="DRAM"))

    # ------------------------------------------------ persistent tiles
    x = persist.tile([P, DT, TL], f32, name="x")          # x master
    xb = persist.tile([P, DT, TL], bf16, name="xb")       # bf16 mirror
    y = persist.tile([P, DT, TL], f32, name="y")          # x + sublayer(x)
    qT = persist.tile([P, DT, TL], bf16, name="qT")
    kloc = persist.tile([P, DT, TL], bf16, name="kloc")   # local K^T
    vT = persist.tile([P, DT, TL], bf16, name="vT")       # local V^T
    vloc = persist.tile([P, 2, H, HD], bf16, name="vloc")  # local V tok-major
    kT = persist.tile([P, DT, S], bf16, name="kT")        # full K^T
    vsb = persist.tile([P, KT, H, HD + 1], bf16, name="vsb")  # V | ones
    attnT = persist.tile([P, DT, TL], bf16, name="attnT")
    attnTB = persist.tile([64, DT, TL], bf16, name="attnTB")  # odd heads
    h = persist.tile([P, FT, TL], bf16, name="h")         # ffn hidden
    identb = persist.tile([P, P], bf16, name="identb")
    onesP = persist.tile([P, 64], f32, name="onesP")

    make_identity(nc, identb[:])
    nc.vector.memset(onesP[:], 1.0)
    nc.vector.memset(vsb[:, :, :, HD:HD + 1], 1.0)        # ones lane

    # ---------------------------------------- x = x0 (host: emb[seq]+pes)
    nc.sync.dma_start(x[:], io["x0"])
    for k in range(DT):
        nc.vector.tensor_copy(xb[:, k, :], x[:, k, :])

    # ---------------------------------------- batchnorm helpers
    def stats_partial(lbl, stt, mt):
        """channel sum + sumsq of y[:, mt, :] into stt columns."""
        nc.vector.reduce_sum(out=stt[:, mt:mt + 1], in_=y[:, mt, :],
                             axis=mybir.AxisListType.X)
        scr = epool.tile([P, TL], f32, tag="e", name=f"sq{lbl}_{mt}")
        nc.scalar.activation(scr[:], y[:, mt, :], AF.Square,
                             accum_out=stt[:, 8 + mt:9 + mt])

    def batchnorm(lbl, stt, g_sb, be_sb):
        """y -> x (fp32) and xb (bf16), exact global stats via 8KB AR."""
        arin = drin.tile([P, 16], f32, tag="ari", name=f"ari{lbl}")
        arout = drout.tile([P, 16], f32, tag="aro", addr_space="Shared",
                           name=f"aro{lbl}")
        nc.gpsimd.dma_start(arin[:], stt[:])
        nc.gpsimd.collective_compute(
            "AllReduce", ALU.add, replica_groups=ALL8,
            ins=[arin.opt()], outs=[arout.opt()])
        ast = small.tile([P, 16], f32, tag="ast", name=f"ast{lbl}")
        nc.gpsimd.dma_start(ast[:], arout[:])
        mean = small.tile([P, DT], f32, tag="mean", name=f"mean{lbl}")
        nc.vector.tensor_scalar_mul(mean[:], ast[:, 0:8], 1.0 / T)
        msq = small.tile([P, DT], f32, tag="msq", name=f"msq{lbl}")
        nc.vector.tensor_tensor(out=msq[:], in0=mean[:], in1=mean[:],
                                op=ALU.mult)
        veps = small.tile([P, DT], f32, tag="veps", name=f"veps{lbl}")
        nc.vector.scalar_tensor_tensor(out=veps[:], in0=ast[:, 8:16],
                                       scalar=1.0 / T, in1=msq[:],
                                       op0=ALU.mult, op1=ALU.subtract)
        nc.vector.tensor_scalar_add(veps[:], veps[:], EPS)
        rec = small.tile([P, DT], f32, tag="rec", name=f"rec{lbl}")
        nc.vector.reciprocal(rec[:], veps[:])
        rstd = small.tile([P, DT], f32, tag="rstd", name=f"rstd{lbl}")
        nc.scalar.sqrt(rstd[:], rec[:])
        sc = small.tile([P, DT], f32, tag="sc", name=f"sc{lbl}")
        nc.vector.tensor_tensor(out=sc[:], in0=g_sb[:], in1=rstd[:],
                                op=ALU.mult)
        sh = small.tile([P, DT], f32, tag="sh", name=f"sh{lbl}")
        nc.vector.tensor_tensor(out=sh[:], in0=mean[:], in1=sc[:], op=ALU.mult)
        nc.vector.tensor_tensor(out=sh[:], in0=be_sb[:], in1=sh[:],
                                op=ALU.subtract)
        for k in range(DT):
            nc.scalar.activation(x[:, k, :], y[:, k, :], AF.Identity,
                                 bias=sh[:, k:k + 1], scale=sc[:, k:k + 1])
            nc.vector.tensor_copy(xb[:, k, :], x[:, k, :])

    # ---------------------------------------- layers
    for l in range(n_layers):
        # ---- per-layer small params
        bq_sb = small.tile([P, DT], f32, tag="bq", name=f"bq{l}")
        bk_sb = small.tile([P, DT], f32, tag="bk", name=f"bk{l}")
        bv_sb = small.tile([P, DT], f32, tag="bv", name=f"bv{l}")
        b1_sb = small.tile([P, FT], f32, tag="b1", name=f"b1{l}")
        g1_sb = small.tile([P, DT], f32, tag="g1", name=f"g1{l}")
        be1_sb = small.tile([P, DT], f32, tag="be1", name=f"be1{l}")
        g2_sb = small.tile([P, DT], f32, tag="g2", name=f"g2{l}")
        be2_sb = small.tile([P, DT], f32, tag="be2", name=f"be2{l}")
        for nm, t_ in [("bq", bq_sb), ("bk", bk_sb), ("bv", bv_sb),
                       ("b1", b1_sb), ("g1", g1_sb), ("be1", be1_sb),
                       ("g2", g2_sb), ("be2", be2_sb)]:
            nc.sync.dma_start(t_[:], io[nm][l])

        # ---- QKV projections (local tokens, all heads); K first, V, then Q
        # so the K AllGather flies during V/Q compute, V's during Q.
        def proj(wname, dst, bias):
            for half in range(2):
                wcb = wc8.tile([P, DT, 512], bf16, tag="wc",
                               name=f"{wname}{l}_{half}")
                nc.sync.dma_start(wcb[:], io[wname][l, half])
                for m in range(4):
                    mt = half * 4 + m
                    psq = ps.tile([P, TL], f32, tag="mm",
                                  name=f"p{wname}{l}_{half}_{m}")
                    for k in range(DT):
                        nc.tensor.matmul(psq[:], wcb[:, k, ts(m, P)],
                                         xb[:, k, :],
                                         start=(k == 0), stop=(k == DT - 1))
                    nc.scalar.activation(dst[:, mt, :], psq[:], AF.Identity,
                                         bias=bias[:, mt:mt + 1])

        proj("wk", kloc, bk_sb)
        aginK = drin.tile([P, DT * TL], bf16, tag="agik", name=f"agik{l}")
        agoK = drout.tile([2, P, DT * TL], bf16, tag="agok", name=f"agok{l}")
        nc.gpsimd.dma_start(
            aginK[:].rearrange("p (k t) -> p k t", k=DT), kloc[:])
        nc.gpsimd.collective_compute(
            "AllGather", ALU.bypass, replica_groups=PAIRS,
            ins=[aginK.opt()], outs=[agoK.opt()])

        proj("wv", vT, bv_sb)
        # local V^T -> token-major V
        for tt in range(2):
            for k in range(DT):
                ptile = pst.tile([P, P], bf16, tag="tp", name=f"vt{l}_{tt}_{k}")
                nc.tensor.transpose(ptile[:], vT[:, k, ts(tt, P)], identb[:])
                nc.vector.tensor_copy(
                    vloc[:, tt, 2 * k:2 * k + 2, :],
                    ptile[:].rearrange("p (h e) -> p h e", h=2))
        aginV = drin.tile([P, DT * TL], bf16, tag="agiv", name=f"agiv{l}")
        agoV = drout.tile([2, P, DT * TL], bf16, tag="agov", name=f"agov{l}")
        nc.gpsimd.dma_start(
            aginV[:].rearrange("p (a h e) -> p a h e", a=2, h=H), vloc[:])
        nc.gpsimd.collective_compute(
            "AllGather", ALU.bypass, replica_groups=PAIRS,
            ins=[aginV.opt()], outs=[agoV.opt()])

        proj("wq", qT, bq_sb)

        for r in range(2):
            nc.gpsimd.dma_start(
                kT[:, :, r * TL:(r + 1) * TL],
                agoK[r].rearrange("p (k t) -> p k t", k=DT))
            nc.gpsimd.dma_start(
                vsb[:, 2 * r:2 * r + 2, :, 0:HD],
                agoV[r].rearrange("p (a h e) -> p a h e", a=2, h=H))

        # ---- attention, software-pipelined across heads so each engine's
        # per-head stage overlaps the next head's matmuls
        def head_scores(hd_):
            hb, kk = (hd_ % 2) * HD, hd_ // 2
            ets = []
            for kt in range(KT):
                pss = ps.tile([P, TL], f32, tag="mm", name=f"ps{l}_{hd_}_{kt}")
                nc.tensor.matmul(pss[:], kT[hb:hb + HD, kk, ts(kt, P)],
                                 qT[hb:hb + HD, kk, :], start=True, stop=True)
                et = epool.tile([P, TL], bf16, tag="eb",
                                name=f"et{l}_{hd_}_{kt}")
                nc.scalar.activation(et[:], pss[:], AF.Exp, scale=att_scale)
                ets.append(et)
            return ets

        def head_finish(hd_, ets):
            hb, kk = (hd_ % 2) * HD, hd_ // 2
            psu = pu.tile([P, TL], f32, tag="u", name=f"pu{l}_{hd_}")
            for kt in range(KT):
                nc.tensor.matmul(psu[0:HD + 1, :], vsb[:, kt, hd_, :],
                                 ets[kt][:], start=(kt == 0),
                                 stop=(kt == KT - 1))
            rsb = epool.tile([P, TL], f32r, tag="er", name=f"rs{l}_{hd_}")
            with nc.allow_low_precision(reason="f32r == f32 precision"):
                nc.vector.reciprocal(rsb[HD:HD + 1, :], psu[HD:HD + 1, :])
            psr = pu.tile([P, TL], f32, tag="u", name=f"pr{l}_{hd_}")
            nc.tensor.matmul(psr[0:HD, :], _r(onesP[HD:HD + 1, :]),
                             rsb[HD:HD + 1, :], start=True, stop=True)
            usb = epool.tile([P, TL], f32, tag="e", name=f"us{l}_{hd_}")
            nc.scalar.copy(usb[0:HD, :], psu[0:HD, :])
            dst = (attnT[0:HD, kk, :] if hd_ % 2 == 0 else attnTB[:, kk, :])
            nc.vector.tensor_tensor(out=dst, in0=usb[0:HD, :],
                                    in1=psr[0:HD, :], op=ALU.mult)
            if hd_ % 2 == 1:   # odd heads land at partitions 64..128 via DMA
                nc.sync.dma_start(attnT[HD:P, kk, :], attnTB[:, kk, :])

        prev = None
        for hd_ in range(H):
            ets = head_scores(hd_)
            if prev is not None:
                head_finish(*prev)
            prev = (hd_, ets)
        head_finish(*prev)

        # ---- Wo + residual -> y, stats partials inline
        stt1 = small.tile([P, 16], f32, tag="stt", name=f"stt_a{l}")
        for half in range(2):
            woc = wc8.tile([P, DT, 512], bf16, tag="wc", name=f"wo{l}_{half}")
            nc.sync.dma_start(woc[:], io["wo"][l, half])
            for m in range(4):
                mt = half * 4 + m
                ps2 = ps.tile([P, TL], f32, tag="mm", name=f"o{l}_{half}_{m}")
                for k in range(DT):
                    nc.tensor.matmul(ps2[:], woc[:, k, ts(m, P)],
                                     attnT[:, k, :],
                                     start=(k == 0), stop=(k == DT - 1))
                nc.vector.tensor_tensor(out=y[:, mt, :], in0=ps2[:],
                                        in1=x[:, mt, :], op=ALU.add)
                stats_partial(f"a{l}", stt1, mt)

        if DEBUG_TAPS and l == 0:
            nc.sync.dma_start(io["dbg_q"], qT[:])
            nc.sync.dma_start(io["dbg_k"], kT[:])
            nc.sync.dma_start(io["dbg_v"], vsb[:])
            nc.sync.dma_start(io["dbg_attn"], attnT[:])
            nc.sync.dma_start(io["dbg_y1"], y[:])

        # ---- BN1 -> x, xb
        batchnorm(f"a{l}", stt1, g1_sb, be1_sb)
        if DEBUG_TAPS and l == 0:
            nc.sync.dma_start(io["dbg_x2"], x[:])

        # ---- FFN1: h = relu(W1^T x + b1)
        for c in range(8):
            w1c = wc8.tile([P, DT, 512], bf16, tag="wc", name=f"w1{l}_{c}")
            nc.sync.dma_start(w1c[:], io["w1"][l, c])
            for m in range(4):
                mt = c * 4 + m
                ps1 = ps.tile([P, TL], f32, tag="mm", name=f"f{l}_{c}_{m}")
                for k in range(DT):
                    nc.tensor.matmul(ps1[:], w1c[:, k, ts(m, P)], xb[:, k, :],
                                     start=(k == 0), stop=(k == DT - 1))
                nc.scalar.activation(h[:, mt, :], ps1[:], AF.Relu,
                                     bias=b1_sb[:, mt:mt + 1])

        # ---- FFN2 + residual -> y, stats partials inline
        stt2 = small.tile([P, 16], f32, tag="stt", name=f"stt_f{l}")
        for m in range(DT):
            w2c = wc32.tile([P, FT, P], bf16, tag="wc2", name=f"w2{l}_{m}")
            nc.sync.dma_start(w2c[:], io["w2"][l, m])
            ps2 = pu.tile([P, TL], f32, tag="u", name=f"g{l}_{m}")
            for k in range(FT):
                nc.tensor.matmul(ps2[:], w2c[:, k, :], h[:, k, :],
                                 start=(k == 0), stop=(k == FT - 1))
            nc.vector.tensor_tensor(out=y[:, m, :], in0=ps2[:],
                                    in1=x[:, m, :], op=ALU.add)
            stats_partial(f"f{l}", stt2, m)

        # ---- BN2 -> x, xb
        batchnorm(f"f{l}", stt2, g2_sb, be2_sb)

    # ---------------------------------------- output x -> [D, TL]
    nc.sync.dma_start(io["out"].rearrange("(k p) t -> p k t", p=P), x[:])
    st.close()


# ================================================================ host side

def make_in_maps(inputs):
    import ml_dtypes
    f = lambda a: np.ascontiguousarray(np.asarray(a), dtype=np.float32)
    b = lambda a: np.ascontiguousarray(np.asarray(a, dtype=np.float32)
                                       .astype(ml_dtypes.bfloat16))
    seq = np.asarray(inputs["sequence"]).astype(np.int64)       # [B, S]
    emb = f(inputs["emb"])
    pes = f(inputs["pes"])
    x0 = emb[seq] + pes[None, :, :]                             # [B, S, D]

    Wq, Wk, Wv = inputs["Wq"], inputs["Wk"], inputs["Wv"]
    Wo, W1, W2 = inputs["Wo"], inputs["W1"], inputs["W2"]

    def chunk8(W, c):
        # [L, D, M] -> [L, c, P, DT, M//c]  with lhsT layout [k*P+p, m]
        W = np.asarray(W, dtype=np.float32)
        Lw, Dw, M = W.shape
        W = W.reshape(Lw, DT, P, c, M // c)
        return np.ascontiguousarray(
            W.transpose(0, 3, 2, 1, 4)).astype(ml_dtypes.bfloat16)

    def chunk_w2(W):
        # [L, DF, D] -> [L, 8, P, FT, P]: chunk m-tiles, k full
        W = np.asarray(W, dtype=np.float32)
        W = W.reshape(L, FT, P, DT, P)
        return np.ascontiguousarray(
            W.transpose(0, 3, 2, 1, 4)).astype(ml_dtypes.bfloat16)

    def vecP(v, n):
        # [L, n*P] -> [L, P, n]
        v = np.asarray(v, dtype=np.float32).reshape(L, n, P)
        return np.ascontiguousarray(v.transpose(0, 2, 1))

    shared = {
        "wq": chunk8(Wq, 2), "wk": chunk8(Wk, 2), "wv": chunk8(Wv, 2),
        "wo": chunk8(Wo, 2), "w1": chunk8(W1, 8), "w2": chunk_w2(W2),
        "bq": vecP(inputs["bq"], DT), "bk": vecP(inputs["bk"], DT),
        "bv": vecP(inputs["bv"], DT), "b1": vecP(inputs["b1"], FT),
        "g1": vecP(inputs["g1"], DT), "be1": vecP(inputs["be1"], DT),
        "g2": vecP(inputs["g2"], DT), "be2": vecP(inputs["be2"], DT),
    }

    in_maps = []
    for c in range(NC):
        bi, hf = c // 2, c % 2
        xs = x0[bi, hf * TL:(hf + 1) * TL, :]                   # [TL, D]
        x0T = np.ascontiguousarray(
            xs.T.reshape(DT, P, TL).transpose(1, 0, 2))          # [P, DT, TL]
        m = {"x0": x0T}
        m.update(shared)
        in_maps.append(m)
    return in_maps


def assemble_output(res):
    out = np.empty((B, S, D), dtype=np.float32)
    for c in range(NC):
        o = np.asarray(res.results[c]["out"])                   # [D, TL]
        bi, hf = c // 2, c % 2
        out[bi, hf * TL:(hf + 1) * TL, :] = o.T
    return out


_CACHE = {}


def _get_module():
    if "nc" not in _CACHE:
        _CACHE["nc"] = build_module()
    return _CACHE["nc"]


def kernel(**inputs):
    from concourse import bass_utils
    nc = _get_module()
    in_maps = make_in_maps(inputs)
    res = bass_utils.run_bass_kernel_spmd(nc, in_maps, list(range(NC)))
    return assemble_output(res)


# revision 21
# speedup vs baseline: 2.7660x; 1.1448x over previous
"""Trainium2 Bass kernel for a 6-layer post-BatchNorm transformer encoder.

Reference model:
  x = emb[seq] + pes                                  # [B,S,D] = [4,512,1024]
  6x: x = BN(x + attn(x)); x = BN(x + ffn(x))
  BN = per-channel batch stats over (B,S), eps=1e-3.

Sharding: data-parallel over tokens across 8 NeuronCores. Core c owns the
256 tokens  [batch c//2, sequence half c%2].  Weights are replicated
(streamed from HBM in bf16, host-converted).  Per layer the only
communication is:
  - a pair AllGather ([[0,1],[2,3],...]) exchanging K^T and token-major V
    (1MB bf16) so each core holds its batch's full 512-key sequence, and
  - two 8KB 8-rank AllReduces for the BatchNorm batch statistics
    (sum / sum-of-squares per channel).
This removes the TP-style [D,T] activation AllReduces entirely.

Numerics: matmul operands bf16 (weights + activation mirrors), PSUM
accumulation fp32, residual/BN arithmetic fp32.  x master kept fp32.
Attention per (head): scores^T = K_h @ Q_h^T (K=64 contraction),
E = exp(scale*scores^T) in bf16 (max-subtraction skipped; scores are O(1)),
U^T = V_h^T @ E^T with denominators from a ones-column appended to V,
normalized by a PE-broadcast reciprocal row.  bo/b2 biases cancel inside
BN and are dropped.  Host does input marshalling only: embedding lookup
(emb[seq]+pes), weight bf16 conversion + chunk-major relayout, output
reassembly.
"""

import os

import numpy as np

import concourse.bass as bass
import concourse.mybir as mybir
import concourse.tile as tile
from concourse import bacc
from concourse.bass import ts
from concourse.masks import make_identity

# ---------------------------------------------------------------- dims
V, D, L, H, B, S = 32000, 1024, 6, 16, 4, 512
HD = D // H            # 64
DF = 4 * D             # 4096
EPS = 1e-3
NC = 8                 # cores
T = B * S              # 2048 tokens (global, for BN stats)
P = 128                # partitions
TL = 256               # local tokens per core
DT = D // P            # 8 d-tiles
FT = DF // P           # 32 ffn hidden tiles
KT = S // P            # 4 key tiles (full sequence)

f32 = mybir.dt.float32
bf16 = mybir.dt.bfloat16
f32r = mybir.dt.float32r
AF = mybir.ActivationFunctionType
ALU = mybir.AluOpType

PAIRS = [[2 * i, 2 * i + 1] for i in range(4)]
ALL8 = [list(range(NC))]

N_LAYERS = int(os.environ.get("TRN_KERNEL_LAYERS", str(L)))
DEBUG_TAPS = os.environ.get("TRN_KERNEL_DEBUG", "0") == "1"


def _r(ap):
    """view an fp32 AP as float32r for full-rate PE matmul"""
    return ap.bitcast(f32r)


def build_module(n_layers=None):
    if n_layers is None:
        n_layers = N_LAYERS
    nc = bacc.Bacc("TRN2", target_bir_lowering=False, debug=False,
                   num_devices=NC)

    dt_ = nc.dram_tensor
    io = {
        "x0": dt_("x0", [P, DT, TL], f32, kind="ExternalInput").ap(),
        # weight chunks, host-prelaid so every chunk DMA is contiguous 1MB
        "wq": dt_("wq", [L, 2, P, DT, 512], bf16, kind="ExternalInput").ap(),
        "wk": dt_("wk", [L, 2, P, DT, 512], bf16, kind="ExternalInput").ap(),
        "wv": dt_("wv", [L, 2, P, DT, 512], bf16, kind="ExternalInput").ap(),
        "wo": dt_("wo", [L, 2, P, DT, 512], bf16, kind="ExternalInput").ap(),
        "w1": dt_("w1", [L, 8, P, DT, 512], bf16, kind="ExternalInput").ap(),
        "w2": dt_("w2", [L, 8, P, FT, P], bf16, kind="ExternalInput").ap(),
        "bq": dt_("bq", [L, P, DT], f32, kind="ExternalInput").ap(),
        "bk": dt_("bk", [L, P, DT], f32, kind="ExternalInput").ap(),
        "bv": dt_("bv", [L, P, DT], f32, kind="ExternalInput").ap(),
        "b1": dt_("b1", [L, P, FT], f32, kind="ExternalInput").ap(),
        "g1": dt_("g1", [L, P, DT], f32, kind="ExternalInput").ap(),
        "be1": dt_("be1", [L, P, DT], f32, kind="ExternalInput").ap(),
        "g2": dt_("g2", [L, P, DT], f32, kind="ExternalInput").ap(),
        "be2": dt_("be2", [L, P, DT], f32, kind="ExternalInput").ap(),
        "out": dt_("out", [D, TL], f32, kind="ExternalOutput").ap(),
    }
    if DEBUG_TAPS:
        for nm, shp, dt in [("dbg_q", [P, DT, TL], bf16),
                            ("dbg_k", [P, DT, S], bf16),
                            ("dbg_v", [P, KT, H, HD + 1], bf16),
                            ("dbg_attn", [P, DT, TL], bf16),
                            ("dbg_y1", [P, DT, TL], f32),
                            ("dbg_x2", [P, DT, TL], f32)]:
            io[nm] = dt_(nm, shp, dt, kind="ExternalOutput").ap()

    with tile.TileContext(nc) as tc:
        _build(tc, n_layers, io)
    nc.compile()
    return nc


def _build(tc, n_layers, io):
    from contextlib import ExitStack
    nc = tc.nc
    att_scale = 1.0 / np.sqrt(HD)

    # ------------------------------------------------ pools
    st = ExitStack()
    persist = st.enter_context(tc.tile_pool(name="persist", bufs=1))
    wc8 = st.enter_context(tc.tile_pool(name="wc8", bufs=5))    # [P,8,512] bf16
    wc32 = st.enter_context(tc.tile_pool(name="wc32", bufs=3))  # [P,32,128] bf16
    small = st.enter_context(tc.tile_pool(name="small", bufs=2))
    epool = st.enter_context(tc.tile_pool(name="epool", bufs=8))
    ps = st.enter_context(tc.tile_pool(name="ps", bufs=4, space="PSUM"))
    pu = st.enter_context(tc.tile_pool(name="pu", bufs=3, space="PSUM"))
    pst = st.enter_context(tc.tile_pool(name="pst", bufs=1, space="PSUM"))
    drin = st.enter_context(tc.tile_pool(name="drin", bufs=2, space="DRAM"))
    drout = st.enter_context(tc.tile_pool(name="drout", bufs=2, space="DRAM"))

    # ------------------------------------------------ persistent tiles
    x = persist.tile([P, DT, TL], f32, name="x")          # x master
    xb = persist.tile([P, DT, TL], bf16, name="xb")       # bf16 mirror
    y = persist.tile([P, DT, TL], f32, name="y")          # x + sublayer(x)
    qT = persist.tile([P, DT, TL], bf16, name="qT")
    kloc = persist.tile([P, DT, TL], bf16, name="kloc")   # local K^T
    vT = persist.tile([P, DT, TL], bf16, name="vT")       # local V^T
    vloc = persist.tile([P, 2, H, HD], bf16, name="vloc")  # local V tok-major
    kT = persist.tile([P, DT, S], bf16, name="kT")        # full K^T
    vsb = persist.tile([P, KT, H, HD + 1], bf16, name="vsb")  # V | ones
    attnT = persist.tile([P, DT, TL], bf16, name="attnT")
    h = persist.tile([P, FT, TL], bf16, name="h")         # ffn hidden
    identb = persist.tile([P, P], bf16, name="identb")

    make_identity(nc, identb[:])
    nc.vector.memset(vsb[:, :, :, HD:HD + 1], 1.0)        # ones lane

    # Dummy collective to absorb the one-time ~100us ncfw/driver warmup
    # while the embedding load and first projections run.
    wrm = persist.tile([P, 16], f32, name="wrm")
    nc.vector.memset(wrm[:], 0.0)
    wrin = drin.tile([P, 16], f32, tag="ari", name="wrin")
    wrout = drout.tile([P, 16], f32, tag="aro", addr_space="Shared",
                       name="wrout")
    nc.gpsimd.dma_start(wrin[:], wrm[:])
    nc.gpsimd.collective_compute(
        "AllReduce", ALU.add, replica_groups=ALL8,
        ins=[wrin.opt()], outs=[wrout.opt()])

    # ---------------------------------------- x = x0 (host: emb[seq]+pes)
    nc.sync.dma_start(x[:], io["x0"])
    for k in range(DT):
        nc.vector.tensor_copy(xb[:, k, :], x[:, k, :])

    # ---------------------------------------- batchnorm helpers
    def stats_partial(lbl, stt, mt):
        """channel sum + sumsq of y[:, mt, :] into stt columns."""
        nc.vector.reduce_sum(out=stt[:, mt:mt + 1], in_=y[:, mt, :],
                             axis=mybir.AxisListType.X)
        scr = epool.tile([P, TL], f32, tag="e", name=f"sq{lbl}_{mt}")
        nc.scalar.activation(scr[:], y[:, mt, :], AF.Square,
                             accum_out=stt[:, 8 + mt:9 + mt])

    def batchnorm(lbl, stt, g_sb, be_sb):
        """y -> x (fp32) and xb (bf16), exact global stats via 8KB AR."""
        arin = drin.tile([P, 16], f32, tag="ari", name=f"ari{lbl}")
        arout = drout.tile([P, 16], f32, tag="aro", addr_space="Shared",
                           name=f"aro{lbl}")
        nc.gpsimd.dma_start(arin[:], stt[:])
        nc.gpsimd.collective_compute(
            "AllReduce", ALU.add, replica_groups=ALL8,
            ins=[arin.opt()], outs=[arout.opt()])
        ast = small.tile([P, 16], f32, tag="ast", name=f"ast{lbl}")
        nc.gpsimd.dma_start(ast[:], arout[:])
        mean = small.tile([P, DT], f32, tag="mean", name=f"mean{lbl}")
        nc.vector.tensor_scalar_mul(mean[:], ast[:, 0:8], 1.0 / T)
        msq = small.tile([P, DT], f32, tag="msq", name=f"msq{lbl}")
        nc.vector.tensor_tensor(out=msq[:], in0=mean[:], in1=mean[:],
                                op=ALU.mult)
        veps = small.tile([P, DT], f32, tag="veps", name=f"veps{lbl}")
        nc.vector.scalar_tensor_tensor(out=veps[:], in0=ast[:, 8:16],
                                       scalar=1.0 / T, in1=msq[:],
                                       op0=ALU.mult, op1=ALU.subtract)
        nc.vector.tensor_scalar_add(veps[:], veps[:], EPS)
        rec = small.tile([P, DT], f32, tag="rec", name=f"rec{lbl}")
        nc.vector.reciprocal(rec[:], veps[:])
        rstd = small.tile([P, DT], f32, tag="rstd", name=f"rstd{lbl}")
        nc.scalar.sqrt(rstd[:], rec[:])
        sc = small.tile([P, DT], f32, tag="sc", name=f"sc{lbl}")
        nc.vector.tensor_tensor(out=sc[:], in0=g_sb[:], in1=rstd[:],
                                op=ALU.mult)
        sh = small.tile([P, DT], f32, tag="sh", name=f"sh{lbl}")
        nc.vector.tensor_tensor(out=sh[:], in0=mean[:], in1=sc[:], op=ALU.mult)
        nc.vector.tensor_tensor(out=sh[:], in0=be_sb[:], in1=sh[:],
                                op=ALU.subtract)
        for k in range(DT):
            nc.scalar.activation(x[:, k, :], y[:, k, :], AF.Identity,
                                 bias=sh[:, k:k + 1], scale=sc[:, k:k + 1])
            nc.vector.tensor_copy(xb[:, k, :], x[:, k, :])

    # ---------------------------------------- layers
    for l in range(n_layers):
        # ---- per-layer small params
        bq_sb = small.tile([P, DT], f32, tag="bq", name=f"bq{l}")
        bk_sb = small.tile([P, DT], f32, tag="bk", name=f"bk{l}")
        bv_sb = small.tile([P, DT], f32, tag="bv", name=f"bv{l}")
        b1_sb = small.tile([P, FT], f32, tag="b1", name=f"b1{l}")
        g1_sb = small.tile([P, DT], f32, tag="g1", name=f"g1{l}")
        be1_sb = small.tile([P, DT], f32, tag="be1", name=f"be1{l}")
        g2_sb = small.tile([P, DT], f32, tag="g2", name=f"g2{l}")
        be2_sb = small.tile([P, DT], f32, tag="be2", name=f"be2{l}")
        for nm, t_ in [("bq", bq_sb), ("bk", bk_sb), ("bv", bv_sb),
                       ("b1", b1_sb), ("g1", g1_sb), ("be1", be1_sb),
                       ("g2", g2_sb), ("be2", be2_sb)]:
            nc.sync.dma_start(t_[:], io[nm][l])

        # ---- QKV projections (local tokens, all heads); K first, V, then Q
        # so the K AllGather flies during V/Q compute, V's during Q.
        def proj(wname, dst, bias):
            for half in range(2):
                wcb = wc8.tile([P, DT, 512], bf16, tag="wc",
                               name=f"{wname}{l}_{half}")
                nc.sync.dma_start(wcb[:], io[wname][l, half])
                for m in range(4):
                    mt = half * 4 + m
                    psq = ps.tile([P, TL], f32, tag="mm",
                                  name=f"p{wname}{l}_{half}_{m}")
                    for k in range(DT):
                        nc.tensor.matmul(psq[:], wcb[:, k, ts(m, P)],
                                         xb[:, k, :],
                                         start=(k == 0), stop=(k == DT - 1))
                    nc.scalar.activation(dst[:, mt, :], psq[:], AF.Identity,
                                         bias=bias[:, mt:mt + 1])

        proj("wk", kloc, bk_sb)
        aginK = drin.tile([P, DT * TL], bf16, tag="agik", name=f"agik{l}")
        agoK = drout.tile([2, P, DT * TL], bf16, tag="agok", name=f"agok{l}")
        nc.gpsimd.dma_start(
            aginK[:].rearrange("p (k t) -> p k t", k=DT), kloc[:])
        nc.gpsimd.collective_compute(
            "AllGather", ALU.bypass, replica_groups=PAIRS,
            ins=[aginK.opt()], outs=[agoK.opt()])

        proj("wv", vT, bv_sb)
        # local V^T -> token-major V
        for tt in range(2):
            for k in range(DT):
                ptile = pst.tile([P, P], bf16, tag="tp", name=f"vt{l}_{tt}_{k}")
                nc.tensor.transpose(ptile[:], vT[:, k, ts(tt, P)], identb[:])
                nc.vector.tensor_copy(
                    vloc[:, tt, 2 * k:2 * k + 2, :],
                    ptile[:].rearrange("p (h e) -> p h e", h=2))
        aginV = drin.tile([P, DT * TL], bf16, tag="agiv", name=f"agiv{l}")
        agoV = drout.tile([2, P, DT * TL], bf16, tag="agov", name=f"agov{l}")
        nc.gpsimd.dma_start(
            aginV[:].rearrange("p (a h e) -> p a h e", a=2, h=H), vloc[:])
        nc.gpsimd.collective_compute(
            "AllGather", ALU.bypass, replica_groups=PAIRS,
            ins=[aginV.opt()], outs=[agoV.opt()])

        proj("wq", qT, bq_sb)

        for r in range(2):
            nc.gpsimd.dma_start(
                kT[:, :, r * TL:(r + 1) * TL],
                agoK[r].rearrange("p (k t) -> p k t", k=DT))
            nc.gpsimd.dma_start(
                vsb[:, 2 * r:2 * r + 2, :, 0:HD],
                agoV[r].rearrange("p (a h e) -> p a h e", a=2, h=H))

        # ---- attention, software-pipelined across heads so each engine's
        # per-head stage overlaps the next head's matmuls
        def head_scores(hd_):
            hb, kk = (hd_ % 2) * HD, hd_ // 2
            ets = []
            for kt in range(KT):
                pss = ps.tile([P, TL], f32, tag="mm", name=f"ps{l}_{hd_}_{kt}")
                nc.tensor.matmul(pss[:], kT[hb:hb + HD, kk, ts(kt, P)],
                                 qT[hb:hb + HD, kk, :], start=True, stop=True)
                et = epool.tile([P, TL], bf16, tag="eb",
                                name=f"et{l}_{hd_}_{kt}")
                nc.scalar.activation(et[:], pss[:], AF.Exp, scale=att_scale)
                ets.append(et)
            return ets

        def head_finish(hd_, ets):
            hb, kk = (hd_ % 2) * HD, hd_ // 2
            psu = pu.tile([P, TL], f32, tag="u", name=f"pu{l}_{hd_}")
            for kt in range(KT):
                nc.tensor.matmul(psu[0:HD + 1, :], vsb[:, kt, hd_, :],
                                 ets[kt][:], start=(kt == 0),
                                 stop=(kt == KT - 1))
            rsb = epool.tile([1, TL], f32, tag="er", name=f"rs{l}_{hd_}")
            nc.vector.reciprocal(rsb[:], psu[HD:HD + 1, :])
            rbc = epool.tile([HD, TL], f32, tag="rb", name=f"rb{l}_{hd_}")
            nc.gpsimd.partition_broadcast(rbc[:], rsb[:])
            nc.vector.tensor_tensor(out=attnT[hb:hb + HD, kk, :],
                                    in0=psu[0:HD, :], in1=rbc[:], op=ALU.mult)

        prev = None
        for hd_ in range(H):
            ets = head_scores(hd_)
            if prev is not None:
                head_finish(*prev)
            prev = (hd_, ets)
        head_finish(*prev)

        # ---- Wo + residual -> y, stats partials inline
        stt1 = small.tile([P, 16], f32, tag="stt", name=f"stt_a{l}")
        for half in range(2):
            woc = wc8.tile([P, DT, 512], bf16, tag="wc", name=f"wo{l}_{half}")
            nc.sync.dma_start(woc[:], io["wo"][l, half])
            for m in range(4):
                mt = half * 4 + m
                ps2 = ps.tile([P, TL], f32, tag="mm", name=f"o{l}_{half}_{m}")
                for k in range(DT):
                    nc.tensor.matmul(ps2[:], woc[:, k, ts(m, P)],
                                     attnT[:, k, :],
                                     start=(k == 0), stop=(k == DT - 1))
                nc.vector.tensor_tensor(out=y[:, mt, :], in0=ps2[:],
                                        in1=x[:, mt, :], op=ALU.add)
                stats_partial(f"a{l}", stt1, mt)

        if DEBUG_TAPS and l == 0:
            nc.sync.dma_start(io["dbg_q"], qT[:])
            nc.sync.dma_start(io["dbg_k"], kT[:])
            nc.sync.dma_start(io["dbg_v"], vsb[:])
            nc.sync.dma_start(io["dbg_attn"], attnT[:])
            nc.sync.dma_start(io["dbg_y1"], y[:])

        # ---- BN1 -> x, xb
        batchnorm(f"a{l}", stt1, g1_sb, be1_sb)
        if DEBUG_TAPS and l == 0:
            nc.sync.dma_start(io["dbg_x2"], x[:])

        # ---- FFN1: h = relu(W1^T x + b1)
        for c in range(8):
            w1c = wc8.tile([P, DT, 512], bf16, tag="wc", name=f"w1{l}_{c}")
            nc.sync.dma_start(w1c[:], io["w1"][l, c])
            for m in range(4):
                mt = c * 4 + m
                ps1 = ps.tile([P, TL], f32, tag="mm", name=f"f{l}_{c}_{m}")
                for k in range(DT):
                    nc.tensor.matmul(ps1[:], w1c[:, k, ts(m, P)], xb[:, k, :],
                                     start=(k == 0), stop=(k == DT - 1))
                nc.scalar.activation(h[:, mt, :], ps1[:], AF.Relu,
                                     bias=b1_sb[:, mt:mt + 1])

        # ---- FFN2 + residual -> y, stats partials inline
        stt2 = small.tile([P, 16], f32, tag="stt", name=f"stt_f{l}")
        for m in range(DT):
            w2c = wc32.tile([P, FT, P], bf16, tag="wc2", name=f"w2{l}_{m}")
            nc.sync.dma_start(w2c[:], io["w2"][l, m])
            ps2 = pu.tile([P, TL], f32, tag="u", name=f"g{l}_{m}")
            for k in range(FT):
                nc.tensor.matmul(ps2[:], w2c[:, k, :], h[:, k, :],
                                 start=(k == 0), stop=(k == FT - 1))
            nc.vector.tensor_tensor(out=y[:, m, :], in0=ps2[:],
                                    in1=x[:, m, :], op=ALU.add)
            stats_partial(f"f{l}", stt2, m)

        # ---- BN2 -> x, xb
        batchnorm(f"f{l}", stt2, g2_sb, be2_sb)

    # ---------------------------------------- output x -> [D, TL]
    nc.sync.dma_start(io["out"].rearrange("(k p) t -> p k t", p=P), x[:])
    st.close()


# ================================================================ host side

def make_in_maps(inputs):
    import ml_dtypes
    f = lambda a: np.ascontiguousarray(np.asarray(a), dtype=np.float32)
    b = lambda a: np.ascontiguousarray(np.asarray(a, dtype=np.float32)
                                       .astype(ml_dtypes.bfloat16))
    seq = np.asarray(inputs["sequence"]).astype(np.int64)       # [B, S]
    emb = f(inputs["emb"])
    pes = f(inputs["pes"])
    x0 = emb[seq] + pes[None, :, :]                             # [B, S, D]

    Wq, Wk, Wv = inputs["Wq"], inputs["Wk"], inputs["Wv"]
    Wo, W1, W2 = inputs["Wo"], inputs["W1"], inputs["W2"]

    def chunk8(W, c):
        # [L, D, M] -> [L, c, P, DT, M//c]  with lhsT layout [k*P+p, m]
        W = np.asarray(W, dtype=np.float32)
        Lw, Dw, M = W.shape
        W = W.reshape(Lw, DT, P, c, M // c)
        return np.ascontiguousarray(
            W.transpose(0, 3, 2, 1, 4)).astype(ml_dtypes.bfloat16)

    def chunk_w2(W):
        # [L, DF, D] -> [L, 8, P, FT, P]: chunk m-tiles, k full
        W = np.asarray(W, dtype=np.float32)
        W = W.reshape(L, FT, P, DT, P)
        return np.ascontiguousarray(
            W.transpose(0, 3, 2, 1, 4)).astype(ml_dtypes.bfloat16)

    def vecP(v, n):
        # [L, n*P] -> [L, P, n]
        v = np.asarray(v, dtype=np.float32).reshape(L, n, P)
        return np.ascontiguousarray(v.transpose(0, 2, 1))

    shared = {
        "wq": chunk8(Wq, 2), "wk": chunk8(Wk, 2), "wv": chunk8(Wv, 2),
        "wo": chunk8(Wo, 2), "w1": chunk8(W1, 8), "w2": chunk_w2(W2),
        "bq": vecP(inputs["bq"], DT), "bk": vecP(inputs["bk"], DT),
        "bv": vecP(inputs["bv"], DT), "b1": vecP(inputs["b1"], FT),
        "g1": vecP(inputs["g1"], DT), "be1": vecP(inputs["be1"], DT),
        "g2": vecP(inputs["g2"], DT), "be2": vecP(inputs["be2"], DT),
    }

    in_maps = []
    for c in range(NC):
        bi, hf = c // 2, c % 2
        xs = x0[bi, hf * TL:(hf + 1) * TL, :]                   # [TL, D]
        x0T = np.ascontiguousarray(
            xs.T.reshape(DT, P, TL).transpose(1, 0, 2))          # [P, DT, TL]
        m = {"x0": x0T}
        m.update(shared)
        in_maps.append(m)
    return in_maps


def assemble_output(res):
    out = np.empty((B, S, D), dtype=np.float32)
    for c in range(NC):
        o = np.asarray(res.results[c]["out"])                   # [D, TL]
        bi, hf = c // 2, c % 2
        out[bi, hf * TL:(hf + 1) * TL, :] = o.T
    return out


_CACHE = {}


def _get_module():
    if "nc" not in _CACHE:
        _CACHE["nc"] = build_module()
    return _CACHE["nc"]


def kernel(**inputs):
    from concourse import bass_utils
    nc = _get_module()
    in_maps = make_in_maps(inputs)
    res = bass_utils.run_bass_kernel_spmd(nc, in_maps, list(range(NC)))
    return assemble_output(res)
